# revision 1
# baseline (speedup 1.0000x reference)
"""BPLoss Trainium2 kernel: 8-core SPMD over the detection (N) axis.

Per core (shard of R=12544 rows = 98 tiles of 128, processed in 49 pairs):
  - per tile, build one-hot O[row,m] = (m == idx_row) on VectorE;
    PE-transpose to OT (bf16)
  - PE gathers G[row,:] = gt_xywh[idx_row,:] via OT @ gtx (bf16, cheap)
  - PE builds the mask correction in PSUM: corr = OT @ LT where
    LT[m,c] = -BIG * (c == gt_label[m]) is a host-precomputed constant
    (bf16 one-hot x {0,-BIG} -> exact f32 accumulation)
  - ScalarE copies corr PSUM->SBUF (two tiles side by side); the pair of
    class-score tiles is DMA-accumulated onto it in one 1 MiB SWDGE
    transfer (accum_op=add), so masked scores materialize with zero
    VectorE cost
  - VectorE does one fused reduce_max pass per pair (the only full-width
    DVE work), ScalarE computes log of the row maxes
  - fused multiply-accumulate reductions produce per-partition partials
Host: shard/pad/layout inputs, sum the 8x[128,2] partials, combine.
"""
import numpy as np
import ml_dtypes
import concourse.bass as bass
import concourse.tile as tile
from concourse import bacc, mybir
from concourse.bass_utils import run_bass_kernel_spmd

N, C, M = 100000, 1024, 128
NCORES = 8
T = 98              # 128-row tiles per core
R = T * 128         # 12544 rows per core
BIG = 1024.0
NBLK = C // 512     # matmul column blocks (PSUM bank = 512 fp32)

f32 = mybir.dt.float32
bf16 = mybir.dt.bfloat16
i32 = mybir.dt.int32
OP = mybir.AluOpType
AF = mybir.ActivationFunctionType
AX = mybir.AxisListType

# packed f32 input layout: [idx | z | r | xywh] along free dim
PF_IDX = 0
PF_Z = PF_IDX + T
PF_R = PF_Z + T
PF_XYWH = PF_R + T
PF_COLS = PF_XYWH + 4 * T
# packed bf16 input layout: [gtx | LT | ident | iotam]
PB_GTX = 0
PB_LT = PB_GTX + 4
PB_ID = PB_LT + C
PB_IOTA = PB_ID + 128
PB_COLS = PB_IOTA + M

# tuning knobs
GROUP_LEN = 2        # 128-row tiles per accumulate-DMA group
SMALL_BUFS = 12
CORR_BUFS = 10
PS_CORR_BUFS = 2
PS_SMALL_BUFS = 2
PREFETCH_PAIRS = 2   # groups whose cs DMA is issued eagerly (plain) to fill the
                     # ramp; corrected afterwards by an SBUF->SBUF accum-DMA
CHUNK_HI = (50, 96, 98)  # epilogue work (lm/partA/partB) chunk boundaries


def build_nc(reps=1, swq=4):
    nc = bacc.Bacc("TRN2", target_bir_lowering=False, debug=False, num_devices=NCORES,
                   num_swdge_queues=swq)
    cs = nc.dram_tensor("cs", [T, 128, C], f32, kind="ExternalInput").ap()
    pf_d = nc.dram_tensor("pf", [128, PF_COLS], f32, kind="ExternalInput").ap()
    pb_d = nc.dram_tensor("pb", [128, PB_COLS], bf16, kind="ExternalInput").ap()
    out = nc.dram_tensor("out", [128, 2], f32, kind="ExternalOutput").ap()

    with tile.TileContext(nc) as tc:
        with (
            tc.tile_pool(name="const", bufs=1) as constp,
            tc.tile_pool(name="corrp", bufs=CORR_BUFS) as corrp,
            tc.tile_pool(name="prefp", bufs=max(PREFETCH_PAIRS, 1)) as prefp,
            tc.tile_pool(name="small", bufs=SMALL_BUFS) as smallp,
            tc.tile_pool(name="psC", bufs=PS_CORR_BUFS, space="PSUM") as psC,
            tc.tile_pool(name="psA", bufs=PS_SMALL_BUFS, space="PSUM") as psA,
            tc.tile_pool(name="psG", bufs=1, space="PSUM") as psG,
        ):
            # ---- constants: two packed DMAs ----
            pf = constp.tile([128, PF_COLS], f32)
            nc.sync.dma_start(out=pf[:], in_=pf_d[:])
            pb = constp.tile([128, PB_COLS], bf16)
            nc.sync.dma_start(out=pb[:], in_=pb_d[:])
            idx_all = pf[:, PF_IDX : PF_IDX + T]
            z_sb = pf[:, PF_Z : PF_Z + T]
            r_sb = pf[:, PF_R : PF_R + T]
            xywh_sb = pf[:, PF_XYWH : PF_XYWH + 4 * T].rearrange(
                "p (t c) -> p t c", c=4
            )
            gtx_bf = pb[:, PB_GTX : PB_GTX + 4]
            LT = pb[:, PB_LT : PB_LT + C]
            ident = pb[:, PB_ID : PB_ID + 128]
            iota_m = pb[:, PB_IOTA : PB_IOTA + M]

            w_sb = constp.tile([128, T], f32)
            nc.vector.tensor_add(w_sb[:], z_sb, r_sb)

            G_all_ps = psG.tile([128, T, 4], f32)   # gathered gt_xywh (PE writes)
            rowmax = constp.tile([128, T], f32)
            out_sb = constp.tile([128, 2], f32)
            nchunk = len(CHUNK_HI)
            acc4 = constp.tile([128, nchunk, 2], f32)
            lm = constp.tile([128, T], f32)
            d_pt = constp.tile([128, T], f32)
            diff = constp.tile([128, T, 4], f32)
            sq = constp.tile([128, T, 4], f32)

            # schedule: groups of tiles; the last two tiles are singles to
            # shorten the post-DMA tail
            GL = GROUP_LEN
            ngrp = (T - 2) // GL
            groups = [(GL * p, GL) for p in range(ngrp)] + [(T - 2, 1), (T - 1, 1)]
            assert ngrp * GL == T - 2

            def tile_chain(t):
                O = smallp.tile([128, M], bf16)
                nc.vector.tensor_scalar(
                    out=O[:], in0=iota_m, scalar1=idx_all[:, t : t + 1],
                    scalar2=None, op0=OP.is_equal,
                )
                OT_ps = psA.tile([128, 128], bf16)
                nc.tensor.transpose(OT_ps[:], O[:], ident)
                OT_sb = smallp.tile([128, 128], bf16)
                nc.vector.tensor_copy(OT_sb[:], OT_ps[:])
                nc.tensor.matmul(
                    G_all_ps[:, t, :], OT_sb[:], gtx_bf, start=True, stop=True
                )
                corr_ps = psC.tile([128, C], f32)
                for b in range(NBLK):
                    sl = slice(b * 512, (b + 1) * 512)
                    nc.tensor.matmul(
                        corr_ps[:, sl], OT_sb[:], LT[:, sl], start=True, stop=True,
                    )
                return corr_ps

            # prefetch pairs: plain cs DMAs issued first (no dependencies);
            # their correction + reduce is emitted a few groups later so the
            # main corr chain stays at the head of every engine queue
            csw_tiles = {}
            for gi in range(PREFETCH_PAIRS):
                t0, glen = groups[gi]
                csw = prefp.tile([128, GROUP_LEN, C], f32)
                nc.sync.dma_start(
                    out=csw[:, 0:glen, :],
                    in_=cs[t0 : t0 + glen].rearrange("a p c -> p a c"),
                )
                csw_tiles[gi] = csw

            for rep in range(reps):
              for gi, (t0, glen) in enumerate(groups):
                prefetch = rep == 0 and gi < PREFETCH_PAIRS
                csr = cs[t0 : t0 + glen].rearrange("a p c -> p a c")
                corrw = corrp.tile([128, GROUP_LEN, C], f32)
                for h in range(glen):
                    corr_ps = tile_chain(t0 + h)
                    nc.scalar.copy(out=corrw[:, h, :], in_=corr_ps[:])
                if prefetch:
                    csw = csw_tiles[gi]
                    nc.gpsimd.dma_start(
                        out=csw[:, 0:glen, :], in_=corrw[:, 0:glen, :],
                        accum_op=OP.add,
                    )
                    red_src = csw
                else:
                    nc.gpsimd.dma_start(
                        out=corrw[:, 0:glen, :], in_=csr, accum_op=OP.add,
                    )
                    red_src = corrw
                nc.vector.reduce_max(
                    rowmax[:, t0 : t0 + glen], red_src[:, 0:glen, :], axis=AX.X
                )

                # interleave epilogue chunk work once its columns are complete
                end = t0 + glen
                for ci, hi in enumerate(CHUNK_HI):
                    lo = 0 if ci == 0 else CHUNK_HI[ci - 1]
                    if not (end >= hi and end - glen < hi):
                        continue
                    # part A: sum (z+r) * ln(rowmax)
                    nc.scalar.activation(
                        out=lm[:, lo:hi], in_=rowmax[:, lo:hi], func=AF.Ln
                    )
                    scr1 = smallp.tile([128, T], f32)
                    nc.vector.scalar_tensor_tensor(
                        out=scr1[:, lo:hi], in0=w_sb[:, lo:hi], scalar=0.0,
                        in1=lm[:, lo:hi], op0=OP.bypass, op1=OP.mult,
                        accum_out=acc4[:, ci, 0:1],
                    )
                    # part B: sum z * ||xywh - gt_xywh[idx]||^2
                    nc.vector.tensor_sub(
                        diff[:, lo:hi, :], xywh_sb[:, lo:hi, :],
                        G_all_ps[:, lo:hi, :],
                    )
                    nc.vector.tensor_mul(
                        sq[:, lo:hi, :], diff[:, lo:hi, :], diff[:, lo:hi, :]
                    )
                    nc.vector.reduce_sum(
                        d_pt[:, lo:hi], sq[:, lo:hi, :], axis=AX.X
                    )
                    scr2 = smallp.tile([128, T], f32)
                    nc.vector.scalar_tensor_tensor(
                        out=scr2[:, lo:hi], in0=z_sb[:, lo:hi], scalar=0.0,
                        in1=d_pt[:, lo:hi], op0=OP.bypass, op1=OP.mult,
                        accum_out=acc4[:, ci, 1:2],
                    )

            # ---- final combine ----
            nc.vector.reduce_sum(out_sb[:, 0:1], acc4[:, :, 0], axis=AX.X)
            nc.vector.reduce_sum(out_sb[:, 1:2], acc4[:, :, 1], axis=AX.X)
            nc.sync.dma_start(out=out[:], in_=out_sb[:])

    nc.compile()
    return nc


def make_in_maps(class_scores, xywh, z, r, nearest_gt_idx, gt_class_labels, gt_xywh):
    cs = np.ascontiguousarray(np.asarray(class_scores, dtype=np.float32))
    xywh = np.ascontiguousarray(np.asarray(xywh, dtype=np.float32))
    z = np.ascontiguousarray(np.asarray(z, dtype=np.float32))
    r = np.ascontiguousarray(np.asarray(r, dtype=np.float32))
    idx = np.asarray(nearest_gt_idx).astype(np.int32)
    gtl = np.asarray(gt_class_labels).astype(np.int64)
    gtx = np.asarray(gt_xywh, dtype=np.float32)

    # packed bf16 constants: [gtx | LT | ident]
    pb = np.zeros((128, PB_COLS), dtype=ml_dtypes.bfloat16)
    pb[:, PB_GTX : PB_GTX + 4] = gtx.astype(ml_dtypes.bfloat16)
    pb[:, PB_LT : PB_LT + C] = (
        -BIG * (gtl[:, None] == np.arange(C)[None, :])
    ).astype(ml_dtypes.bfloat16)
    pb[:, PB_ID : PB_ID + 128] = np.eye(128, dtype=ml_dtypes.bfloat16)
    pb[:, PB_IOTA : PB_IOTA + M] = np.arange(M, dtype=np.float32)[None, :].astype(
        ml_dtypes.bfloat16
    )

    in_maps = []
    for c in range(NCORES):
        lo, hi = c * R, (c + 1) * R
        if hi <= N:
            cs_s, xywh_s, z_s, r_s, idx_s = (
                cs[lo:hi], xywh[lo:hi], z[lo:hi], r[lo:hi], idx[lo:hi],
            )
        else:
            n_real = N - lo
            cs_s = np.ones((R, C), dtype=np.float32)
            cs_s[:n_real] = cs[lo:]
            xywh_s = np.zeros((R, 4), dtype=np.float32)
            xywh_s[:n_real] = xywh[lo:]
            z_s = np.zeros(R, dtype=np.float32)
            z_s[:n_real] = z[lo:]
            r_s = np.zeros(R, dtype=np.float32)
            r_s[:n_real] = r[lo:]
            idx_s = np.zeros(R, dtype=np.int32)
            idx_s[:n_real] = idx[lo:]
        pf = np.empty((128, PF_COLS), dtype=np.float32)
        pf[:, PF_IDX : PF_IDX + T] = idx_s.reshape(T, 128).T
        pf[:, PF_Z : PF_Z + T] = z_s.reshape(T, 128).T
        pf[:, PF_R : PF_R + T] = r_s.reshape(T, 128).T
        pf[:, PF_XYWH : PF_XYWH + 4 * T] = (
            xywh_s.reshape(T, 128, 4).transpose(1, 0, 2).reshape(128, 4 * T)
        )
        in_maps.append({
            "cs": cs_s.reshape(T, 128, C),
            "pf": pf,
            "pb": pb,
        })
    return in_maps


def combine_outputs(outs):
    """outs: list of [128, 2] per-core partials -> final [1] float32."""
    partA = float(sum(o[:, 0].astype(np.float64).sum() for o in outs))
    partB = float(sum(o[:, 1].astype(np.float64).sum() for o in outs))
    with np.errstate(over="ignore", under="ignore"):
        tps = np.exp(-partB)
    val = -partA + tps
    return np.array([val], dtype=np.float32)


_NC_CACHE = None


def get_nc():
    global _NC_CACHE
    if _NC_CACHE is None:
        _NC_CACHE = build_nc()
    return _NC_CACHE


def kernel(**inputs) -> np.ndarray:
    nc = get_nc()
    in_maps = make_in_maps(**inputs)
    res = run_bass_kernel_spmd(nc, in_maps, core_ids=list(range(NCORES)))
    return combine_outputs([res.results[c]["out"] for c in range(NCORES)])



# revision 7
# speedup vs baseline: 260.8326x; 260.8326x over previous
"""BPLoss Trainium2 kernel: 8-core SPMD over the detection (N) axis.

v3 design (replaces the SWDGE accumulate-DMA scheme, which ran at ~0.5x
HBM bandwidth on silicon due to the read-modify-write CCE path):

Per core (shard of R=12544 rows; partition p owns rows p*98..p*98+97, so
each group DMA reads one contiguous 28 KiB run per partition):
  - 14 groups x [128, 7, 1024] f32 plain HWDGE DMAs (3.5 MiB each) on
    the sync queue stream class_scores at line rate
  - masking, per 128-row tile, in place:
      DVE tiles:  masked = (iota != label) * cs   (one fused
                  scalar_tensor_tensor; label is a per-partition scalar)
      Pool tiles: ScalarE builds v = Relu(BIG - BIG*Square(iota-label))
                  (= BIG one-hot at the label column) in two activation
                  passes, GpSimd subtracts it from cs (its only fast op
                  class is tensor_tensor arithmetic)
    Tiles 0..DVE_APPLY-1 of each group go to DVE, the rest to GpSimd, so
    the DVE keeps headroom for the grouped reduce_max
  - one reduce_max per group produces 7 row-max columns at once
  - epilogue: Ln on ScalarE, fused multiply-accumulate dot products for
    sum((z+r)*log_max) and sum(z*||xywh - gt_xywh[idx]||^2)
Host: gathers the tiny gt tables per row (labels, gt_xywh[idx]), shards,
pads core 7, sums the 8x[128,2] partials, combines -A + exp(-B).
"""
import numpy as np
import concourse.bass as bass
import concourse.tile as tile
from concourse import bacc, mybir
from concourse.bass_utils import run_bass_kernel_spmd

N, C, M = 100000, 1024, 128
NCORES = 8
T = 98              # 128-row tiles per core
R = T * 128         # 12544 rows per core
G = 7               # tiles per DMA group
NG = T // G         # 14 groups
DVE_APPLY = 2       # leading tiles of each group masked on DVE; rest GpSimd
CS_BUFS = 3
V_BUFS = 8
BIG = 1024.0

f32 = mybir.dt.float32
OP = mybir.AluOpType
AF = mybir.ActivationFunctionType
AX = mybir.AxisListType

# packed f32 per-row tables: [label | -label | z | r | xywh | g | iota | BIG]
PF_LAB = 0
PF_NLAB = T
PF_Z = 2 * T
PF_R = 3 * T
PF_XYWH = 4 * T
PF_G = 8 * T
PF_IOTA = 12 * T
PF_BIGC = 12 * T + C
PF_COLS = 12 * T + C + 1


def build_nc(reps=1):
    nc = bacc.Bacc("TRN2", target_bir_lowering=False, debug=False,
                   num_devices=NCORES)
    cs_d = nc.dram_tensor("cs", [128, T * C], f32, kind="ExternalInput").ap()
    pf_d = nc.dram_tensor("pf", [128, PF_COLS], f32, kind="ExternalInput").ap()
    out_d = nc.dram_tensor("out", [128, 2], f32, kind="ExternalOutput").ap()

    with tile.TileContext(nc) as tc:
        with (
            tc.tile_pool(name="const", bufs=1) as constp,
            tc.tile_pool(name="csp", bufs=CS_BUFS) as csp,
            tc.tile_pool(name="vp", bufs=V_BUFS) as vp,
        ):
            pf = constp.tile([128, PF_COLS], f32)
            nc.sync.dma_start(out=pf[:], in_=pf_d[:])
            lab = pf[:, PF_LAB : PF_LAB + T]
            nlab = pf[:, PF_NLAB : PF_NLAB + T]
            z_sb = pf[:, PF_Z : PF_Z + T]
            r_sb = pf[:, PF_R : PF_R + T]
            xywh_sb = pf[:, PF_XYWH : PF_XYWH + 4 * T].rearrange(
                "p (t c) -> p t c", c=4
            )
            g_sb = pf[:, PF_G : PF_G + 4 * T].rearrange("p (t c) -> p t c", c=4)
            iota = pf[:, PF_IOTA : PF_IOTA + C]
            bigc = pf[:, PF_BIGC : PF_BIGC + 1]

            w_sb = constp.tile([128, T], f32)
            nc.vector.tensor_add(w_sb[:], z_sb, r_sb)
            rowmax = constp.tile([128, T], f32)
            lm = constp.tile([128, T], f32)
            out_sb = constp.tile([128, 2], f32)
            scr = constp.tile([128, T], f32)
            scr2 = constp.tile([128, T], f32)
            diff = constp.tile([128, T, 4], f32)
            dsum = constp.tile([128, T], f32)

            for rep in range(reps):
                for g in range(NG):
                    t0 = g * G
                    csw = csp.tile([128, G, C], f32)
                    nc.sync.dma_start(
                        out=csw[:],
                        in_=cs_d[:, t0 * C : (t0 + G) * C].rearrange(
                            "p (a c) -> p a c", c=C
                        ),
                    )
                    for h in range(G):
                        t = t0 + h
                        if h < DVE_APPLY:
                            nc.vector.scalar_tensor_tensor(
                                out=csw[:, h, :], in0=iota,
                                scalar=lab[:, t : t + 1],
                                in1=csw[:, h, :],
                                op0=OP.not_equal, op1=OP.mult,
                            )
                        else:
                            v = vp.tile([128, C], f32)
                            nc.scalar.activation(
                                out=v[:], in_=iota, func=AF.Square,
                                scale=1.0, bias=nlab[:, t : t + 1],
                            )
                            nc.scalar.activation(
                                out=v[:], in_=v[:], func=AF.Relu,
                                scale=-BIG, bias=bigc,
                            )
                            nc.gpsimd.tensor_tensor(
                                out=csw[:, h, :], in0=csw[:, h, :],
                                in1=v[:], op=OP.subtract,
                            )
                    nc.vector.reduce_max(
                        rowmax[:, t0 : t0 + G], csw[:], axis=AX.X
                    )

                # epilogue: partial sums
                nc.scalar.activation(out=lm[:], in_=rowmax[:], func=AF.Ln)
                nc.vector.scalar_tensor_tensor(
                    out=scr[:], in0=w_sb[:], scalar=0.0, in1=lm[:],
                    op0=OP.bypass, op1=OP.mult, accum_out=out_sb[:, 0:1],
                )
                nc.vector.tensor_sub(diff[:], xywh_sb, g_sb)
                nc.vector.tensor_mul(diff[:], diff[:], diff[:])
                nc.vector.reduce_sum(dsum[:], diff[:], axis=AX.X)
                nc.vector.scalar_tensor_tensor(
                    out=scr2[:], in0=z_sb, scalar=0.0, in1=dsum[:],
                    op0=OP.bypass, op1=OP.mult, accum_out=out_sb[:, 1:2],
                )
            nc.sync.dma_start(out=out_d[:], in_=out_sb[:])

    nc.compile()
    return nc


def make_in_maps(class_scores, xywh, z, r, nearest_gt_idx, gt_class_labels, gt_xywh):
    cs = np.ascontiguousarray(np.asarray(class_scores, dtype=np.float32))
    xywh = np.ascontiguousarray(np.asarray(xywh, dtype=np.float32))
    z = np.ascontiguousarray(np.asarray(z, dtype=np.float32))
    r = np.ascontiguousarray(np.asarray(r, dtype=np.float32))
    idx = np.asarray(nearest_gt_idx).astype(np.int64)
    labels = np.asarray(gt_class_labels).astype(np.float32)[idx]       # [N]
    gx = np.asarray(gt_xywh, dtype=np.float32)[idx]                    # [N,4]

    iota_row = np.arange(C, dtype=np.float32)[None, :]
    in_maps = []
    for c in range(NCORES):
        lo, hi = c * R, (c + 1) * R
        if hi <= N:
            cs_s = cs[lo:hi]
            lab_s, z_s, r_s = labels[lo:hi], z[lo:hi], r[lo:hi]
            xywh_s, gx_s = xywh[lo:hi], gx[lo:hi]
        else:
            n_real = N - lo
            cs_s = np.ones((R, C), dtype=np.float32)
            cs_s[:n_real] = cs[lo:]
            lab_s = np.zeros(R, np.float32); lab_s[:n_real] = labels[lo:]
            z_s = np.zeros(R, np.float32); z_s[:n_real] = z[lo:]
            r_s = np.zeros(R, np.float32); r_s[:n_real] = r[lo:]
            xywh_s = np.zeros((R, 4), np.float32); xywh_s[:n_real] = xywh[lo:]
            gx_s = np.zeros((R, 4), np.float32); gx_s[:n_real] = gx[lo:]
        pf = np.empty((128, PF_COLS), dtype=np.float32)
        pf[:, PF_LAB : PF_LAB + T] = lab_s.reshape(128, T)
        pf[:, PF_NLAB : PF_NLAB + T] = -lab_s.reshape(128, T)
        pf[:, PF_Z : PF_Z + T] = z_s.reshape(128, T)
        pf[:, PF_R : PF_R + T] = r_s.reshape(128, T)
        pf[:, PF_XYWH : PF_XYWH + 4 * T] = xywh_s.reshape(128, 4 * T)
        pf[:, PF_G : PF_G + 4 * T] = gx_s.reshape(128, 4 * T)
        pf[:, PF_IOTA : PF_IOTA + C] = iota_row
        pf[:, PF_BIGC] = BIG
        in_maps.append({"cs": cs_s.reshape(128, T * C), "pf": pf})
    return in_maps


def combine_outputs(outs):
    """outs: list of [128, 2] per-core partials -> final [1] float32."""
    partA = float(sum(o[:, 0].astype(np.float64).sum() for o in outs))
    partB = float(sum(o[:, 1].astype(np.float64).sum() for o in outs))
    with np.errstate(over="ignore", under="ignore"):
        tps = np.exp(-partB)
    val = -partA + tps
    return np.array([val], dtype=np.float32)


_NC_CACHE = None


def get_nc():
    global _NC_CACHE
    if _NC_CACHE is None:
        _NC_CACHE = build_nc()
    return _NC_CACHE


def kernel(**inputs) -> np.ndarray:
    nc = get_nc()
    in_maps = make_in_maps(**inputs)
    res = run_bass_kernel_spmd(nc, in_maps, core_ids=list(range(NCORES)))
    return combine_outputs([res.results[c]["out"] for c in range(NCORES)])
